# revision 5
# baseline (speedup 1.0000x reference)
"""ChebyKAN layer on 8 Trainium2 NeuronCores.

y[b,o] = sum_{i,d} T_d(tanh(x[b,i])) * coeffs[i,o,d]

The Chebyshev basis is re-parameterized (exact 9x9 linear transform of the
coefficients on host) into products of Chebyshev values built with ACT squares
and DVE fused ops from t = tanh(x):
  G0=1, G1=t, G2=t^2, G3q=(G2-3/4)t=T3/4, G4=(2G2-1)^2=T2^2,
  G5q=(G4-1/2)t=(T5+T3)/4, G6=(4G3q)^2=T3^2, G7q=(G6-1/2)t=(T7+T5)/4,
  G8=(2G4-1)^2=T4^2
G0 contributes a constant bias[o] = sum_i C'[i,o,0], computed on host and
added during PSUM eviction -- only levels 1..8 hit the PE.

Matmul orientation: stationary lhsT = coeff block [i128, o128] (bf16, FWL
weight loads hidden behind streaming), moving rhs = basis tile [i128, b512]
(bf16), PSUM out [o128, b512] fp32.  Per (quarter, ob) one psum bank
accumulates 64 MMs (8 levels x 8 i-blocks).  Batch is processed in quarters
of 512 rows so all 64 basis tiles of a quarter stay SBUF-resident; the next
quarter's basis is produced by ACT/DVE while the PE streams the current one.
Output is produced transposed ([O, B_core]) and untransposed on host.

Sharding: data-parallel over batch (2048 rows/core), coeffs replicated.
"""

import numpy as np
import concourse.mybir as mybir
import concourse.tile as tile
from concourse import bacc
from concourse.bass_utils import run_bass_kernel_spmd

B, I, O, D1 = 16384, 1024, 1024, 9
CORES = 8
BC = B // CORES            # 2048 batch rows per core
P = 128
Q = 512                    # batch rows per quarter
N_Q = BC // Q              # 4 quarters
IB = I // P                # 8 i-blocks
OB = O // P                # 8 o-blocks
# basis production/consumption order within an i-block (respects deps)
LEV_ORDER = [1, 2, 4, 3, 6, 5, 7, 8]

F32 = mybir.dt.float32
BF16 = mybir.dt.bfloat16
AF = mybir.ActivationFunctionType
OP = mybir.AluOpType

_CACHE = {}
_last_in_maps = None

# G_k = sum_d M[k,d] T_d  (exact); host solves M^T C' = C
_M = np.zeros((9, 9))
_M[0, 0] = 1; _M[1, 1] = 1
_M[2, 0] = .5; _M[2, 2] = .5
_M[3, 3] = .25
_M[4, 0] = .5; _M[4, 4] = .5
_M[5, 3] = .25; _M[5, 5] = .25
_M[6, 0] = .5; _M[6, 6] = .5
_M[7, 5] = .25; _M[7, 7] = .25
_M[8, 0] = .5; _M[8, 8] = .5
_A = np.linalg.inv(_M.T)


def _emit_basis(nc, xp, tp, gp, xt_d, neg1, q, rep):
    """Produce the 64 bf16 basis tiles [128, Q] for quarter q."""
    g = {}
    for ib in range(IB):
        sfx = f"{rep}_{q}_{ib}"
        xq = xp.tile([P, Q], F32, tag="xq", name=f"xq{sfx}")
        nc.sync.dma_start(xq[:], xt_d[ib * P:(ib + 1) * P, q * Q:(q + 1) * Q])
        # fp32 chain (t, G2, G4, G3 feed later levels; bf16 only at MM inputs)
        t32 = tp.tile([P, Q], F32, tag="t32", name=f"t32{sfx}")
        g232 = tp.tile([P, Q], F32, tag="g232", name=f"g232{sfx}")
        g432 = tp.tile([P, Q], F32, tag="g432", name=f"g432{sfx}")
        g332 = tp.tile([P, Q], F32, tag="g332", name=f"g332{sfx}")

        def mmt(lev):
            w = gp.tile([P, Q], BF16, tag=f"g{lev}_{ib}", name=f"g{lev}_{ib}_{rep}_{q}")
            g.setdefault(lev, {})[ib] = w
            return w

        nc.scalar.activation(t32[:], xq[:], AF.Tanh)
        nc.vector.tensor_copy(mmt(1)[:], t32[:])
        nc.scalar.activation(g232[:], t32[:], AF.Square)
        nc.vector.tensor_copy(mmt(2)[:], g232[:])
        nc.scalar.activation(g432[:], g232[:], AF.Square, bias=neg1[:], scale=2.0)
        nc.vector.tensor_copy(mmt(4)[:], g432[:])
        nc.vector.scalar_tensor_tensor(
            g332[:], g232[:], 0.75, t32[:], OP.subtract, OP.mult)
        nc.vector.tensor_copy(mmt(3)[:], g332[:])
        nc.scalar.activation(mmt(6)[:], g332[:], AF.Square, scale=4.0)
        nc.vector.scalar_tensor_tensor(
            mmt(5)[:], g432[:], 0.5, t32[:], OP.subtract, OP.mult)
        nc.vector.scalar_tensor_tensor(
            mmt(7)[:], g[6][ib][:], 0.5, t32[:], OP.subtract, OP.mult)
        nc.scalar.activation(mmt(8)[:], g432[:], AF.Square, bias=neg1[:], scale=2.0)
    return g


def _emit_quarter(nc, cp_, op_, pp, c2_d, y_d, bias_sb, g, q, rep):
    """Stream 8 ob groups of 64 accumulating matmuls each for quarter q."""
    for ob in range(OB):
        sfx = f"{rep}_{q}_{ob}"
        cw = cp_.tile([P, 64 * P], BF16, tag="cw", name=f"cw{sfx}")
        nc.sync.dma_start(cw[:], c2_d[ob * P:(ob + 1) * P, :])
        psum = pp.tile([P, Q], F32, tag=f"ps{ob}", name=f"ps{sfx}")
        for ib in range(IB):
            for j, lev in enumerate(LEV_ORDER):
                blk = ib * 8 + j
                nc.tensor.matmul(
                    psum[:],
                    cw[:, blk * P:(blk + 1) * P],
                    g[lev][ib][:],
                    start=(blk == 0),
                    stop=(blk == 63),
                )
        ev = op_.tile([P, Q], F32, tag="ev", name=f"ev{sfx}")
        nc.scalar.activation(ev[:], psum[:], AF.Identity, bias=bias_sb[:, ob:ob + 1])
        nc.sync.dma_start(y_d[ob * P:(ob + 1) * P, q * Q:(q + 1) * Q], ev[:])


def build_nc(reps=1):
    nc = bacc.Bacc("TRN2", target_bir_lowering=False, debug=False, num_devices=CORES)
    xt_d = nc.dram_tensor("xt", [I, BC], F32, kind="ExternalInput")
    # [ob*128 + i_in_blk, (ib*8 + j)*128 + o_in_blk] -- stationary slabs per ob
    c2_d = nc.dram_tensor("c2", [OB * P, 64 * P], BF16, kind="ExternalInput")
    bias_d = nc.dram_tensor("bias", [P, OB], F32, kind="ExternalInput")
    y_d = nc.dram_tensor("y", [O, BC], F32, kind="ExternalOutput")

    with tile.TileContext(nc) as tc:
        with (
            tc.tile_pool(name="xp", bufs=2) as xp,       # x staging
            tc.tile_pool(name="cb", bufs=1) as cb,       # bias
            tc.tile_pool(name="tp", bufs=2) as tp,       # fp32 chain transients
            tc.tile_pool(name="gp", bufs=2) as gp,       # basis tiles (2 quarters)
            tc.tile_pool(name="cp", bufs=2) as cp_,      # coeff slab stream
            tc.tile_pool(name="op", bufs=4) as op_,      # psum eviction staging
            tc.tile_pool(name="pp", bufs=1, space="PSUM") as pp,
        ):
            bias_sb = cb.tile([P, OB], F32, tag="bias")
            nc.sync.dma_start(bias_sb[:], bias_d[:, :])
            neg1 = cb.tile([P, 1], F32, tag="neg1")
            nc.vector.memset(neg1[:], -1.0)

            for rep in range(reps):
                g_next = _emit_basis(nc, xp, tp, gp, xt_d, neg1, 0, rep)
                for q in range(N_Q):
                    g_cur = g_next
                    if q + 1 < N_Q:
                        g_next = _emit_basis(nc, xp, tp, gp, xt_d, neg1, q + 1, rep)
                    _emit_quarter(nc, cp_, op_, pp, c2_d, y_d, bias_sb, g_cur, q, rep)
    nc.compile()
    return nc


def _prep_coeffs(cheby_coeffs):
    cp = np.einsum("ed,iod->ioe", _A, cheby_coeffs.astype(np.float64))
    bias = cp[:, :, 0].sum(axis=0)                       # (O,)
    bias_r = np.ascontiguousarray(
        bias.reshape(OB, P).T.astype(np.float32))        # (128, OB)
    # c2r[ob*128+p, (ib*8+j)*128+c] = cp[ib*128+p, ob*128+c, LEV_ORDER[j]]
    # (partition = i-within-block = contraction dim; free = o-within-block)
    a1 = cp[:, :, LEV_ORDER]                             # (I, O, 8)
    a2 = a1.reshape(IB, P, OB, P, 8)                     # (ib, p, ob, c, j)
    a3 = a2.transpose(2, 1, 0, 4, 3)                     # (ob, p, ib, j, c)
    bf16 = mybir.dt.np(BF16)
    c2r = np.ascontiguousarray(a3.reshape(O, 64 * P).astype(bf16))
    return c2r, bias_r


def kernel(x: np.ndarray, cheby_coeffs: np.ndarray) -> np.ndarray:
    assert x.shape == (B, I) and cheby_coeffs.shape == (I, O, D1)
    if "nc" not in _CACHE:
        _CACHE["nc"] = build_nc()
    nc = _CACHE["nc"]

    xt = np.ascontiguousarray(x.T.astype(np.float32, copy=False))  # (I, B)
    c2r, bias_r = _prep_coeffs(cheby_coeffs)
    in_maps = [
        {
            "xt": np.ascontiguousarray(xt[:, c * BC:(c + 1) * BC]),
            "c2": c2r,
            "bias": bias_r,
        }
        for c in range(CORES)
    ]
    global _last_in_maps
    _last_in_maps = in_maps
    res = run_bass_kernel_spmd(nc, in_maps, core_ids=list(range(CORES)))
    out = np.empty((B, O), np.float32)
    for c in range(CORES):
        out[c * BC:(c + 1) * BC] = res.results[c]["y"].T
    return out
